# revision 1
# baseline (speedup 1.0000x reference)
"""AttentionPooling (ragged graph cross-attention pooling) on 8 TRN2 NeuronCores.

Strategy (SPMD, no collectives):
  * Host assigns 8 whole graphs to each of the 8 cores (serpentine by size),
    sorts each core's graphs by size into 8 "slots".  Slot j has a fixed tile
    count T[j] (shared by all cores, since the instruction stream is shared);
    each graph's edges are placed at its slot offset and zero-padded.
  * Host ships x^T (transposed edge features, bf16) per core + replicated
    weights.  Padding edges give exp(0)=1 in the softmax denominator, which is
    corrected with a host-computed per-slot pad count.
  * Softmax is computed without max-subtraction (scores ~ N(0,1); exp cannot
    overflow fp32) — mathematically identical to the reference's stable form.
  * Scores are linear in x: scores = (x @ w_k) . q  =  x @ Ws where
    Ws[:, (h,s)] = sum_d w_k[:, (h,d)] q[s,h,d] / sqrt(hd).  Ws ([256, 256])
    is host-precomputed from the weights and shipped fused with w_v as one
    [256, 512] operand, so the per-tile device work is:
      [v | sc][e, :] = x @ [w_v | Ws]    (PE, 2 matmuls/tile, N=512)
      ex             = exp(sc)           (ACT, psum->sbuf bf16)
      pooled[(h,s),(h,d)|denom] += ex.T @ [v | 1]  (PE, psum-accum per graph)
  * Per graph: denom -= npad; normalize by 1/denom (DVE); 32x32 block
    transpose (DVE StreamTranspose) to build the [128, (s,half)*8graphs]
    operand P2 for the MLP (w1 needs no permutation in this layout).
  * MLP: h1 = silu(pooled @ w1 + b1) (PE, 4-way tile_position-packed, +ACT),
    out = h1 @ w2 + b2 (PE), emitted as out^T [256, 8] per core; the host
    scatters core outputs into the final [64, 256].
"""

import os
import sys
from contextlib import ExitStack

import numpy as np

for _p in ("/opt/trn_rl_repo",):
    if _p not in sys.path:
        sys.path.append(_p)

import ml_dtypes  # noqa: E402

import concourse.bass as bass  # noqa: E402
import concourse.tile as tile  # noqa: E402
from concourse import mybir  # noqa: E402
from concourse.bass_utils import run_bass_kernel_spmd  # noqa: E402
from concourse.vector_clock import ScopedClock  # noqa: E402

BF16 = ml_dtypes.bfloat16

E, B, H, S, NH, HD = 131072, 64, 256, 32, 8, 32
NCORES = 8
NG = B // NCORES        # graphs (slots) per core
TILE = 128              # edge tile
GROUP = 512             # x^T DMA chunk (4 tiles)
SCALE = 1.0 / float(np.sqrt(HD))

AF = mybir.ActivationFunctionType

# ---------------------------------------------------------------------------
# Walrus workaround: this toolchain's InstDrain accepts only ONE sync wait;
# Tile's kernel-tail drain carries one wait per outstanding semaphore.
# Split it into a chain of single-wait drains.
_MAXW = 1


def _split_drain_and_barrier(self, tick_clock, wait_clock):
    nc = self.nc
    drain_inst = nc.sync.drain()
    wait_clock.add_sem_waits(
        drain_inst.ins, ScopedClock({None: tick_clock.global_clock})
    )
    waits = list(drain_inst.ins.sync_info.on_wait)
    if len(waits) > _MAXW:
        drain_inst.ins.sync_info = mybir.SyncInfo(on_wait=waits[:_MAXW], on_update=[])
        for i in range(_MAXW, len(waits), _MAXW):
            d2 = nc.sync.drain()
            d2.ins.sync_info = mybir.SyncInfo(
                on_wait=waits[i : i + _MAXW], on_update=[]
            )
    nc.all_engine_barrier()
    popped = nc._tile_sem_poison_stack.pop()
    assert popped is self._sem_poison
    nc.clear_and_free_semaphores(list(self.sems.allocated().values()))
    nc.all_engine_barrier()


tile.TileContext._drain_and_barrier = _split_drain_and_barrier

# Engine instructions are capped at 2 sync waits by this walrus (Drain/NoOp
# at 1).  Tile's sem-assignment occasionally emits more.  Hoist the excess
# onto single-wait NoOps inserted just before, on the same engine — the
# engine stalls at the NoOp instead, which is semantically identical.
_WAIT_CAP = {"InstDrain": 1}
_WAIT_CAP_DEFAULT = 1


def _fix_excess_waits(nc):
    n_fixed = 0
    for fn in nc.m.functions:
        for bb in fn.blocks:
            insts = bb.instructions
            out = []
            changed = False
            for inst in insts:
                si = inst.sync_info
                waits = list(si.on_wait) if si is not None else []
                cap = _WAIT_CAP.get(type(inst).__name__, _WAIT_CAP_DEFAULT)
                if len(waits) > cap:
                    changed = True
                    n_fixed += 1
                    excess = waits[: len(waits) - cap]
                    for i, w in enumerate(excess):
                        nop = mybir.InstNoOp(
                            name=f"{inst.name}-hw{i}", ins=[], outs=[]
                        )
                        nop.engine = inst.engine
                        nop.sync_info = mybir.SyncInfo(on_wait=[w], on_update=[])
                        out.append(nop)
                    inst.sync_info = mybir.SyncInfo(
                        on_wait=waits[len(excess) :], on_update=list(si.on_update)
                    )
                out.append(inst)
            if changed:
                bb.instructions = out
    return n_fixed

# ---------------------------------------------------------------------------

_PROGRAM_CACHE: dict[tuple, "bass.Bass"] = {}
LAST_RESULTS = None  # BassKernelResults of the most recent run (for testing)


def _install_ntff_hook_shim():
    """The image's antenv lacks axon_hooks; recreate it so trace=True works."""
    try:
        import types

        import antenv

        if "antenv.axon_hooks" not in sys.modules:
            mod = types.ModuleType("antenv.axon_hooks")
            mod._hook = None

            def set_axon_ntff_profile_hook(h):
                mod._hook = h

            def get_axon_ntff_profile_hook():
                return mod._hook

            mod.set_axon_ntff_profile_hook = set_axon_ntff_profile_hook
            mod.get_axon_ntff_profile_hook = get_axon_ntff_profile_hook
            sys.modules["antenv.axon_hooks"] = mod
            antenv.axon_hooks = mod
        import antenv.axon_hooks as ah

        if ah.get_axon_ntff_profile_hook() is None:
            from trn_agent_boot.trn_boot import _ntff_profile_via_ctypes

            ah.set_axon_ntff_profile_hook(
                _ntff_profile_via_ctypes("/opt/axon/libaxon_pjrt.so")
            )
    except Exception:
        pass


_install_ntff_hook_shim()

# Optional experiment: let walrus double-buffer LDWEIGHTS (default off here).
import concourse.bass_utils as _bass_utils  # noqa: E402

_orig_run_command = _bass_utils.run_command


def _run_command_ldwopt(cmd, **kw):
    if isinstance(cmd, list):
        cmd = [
            "--enable-ldw-opt=true" if c == "--enable-ldw-opt=false" else c
            for c in cmd
        ]
    return _orig_run_command(cmd, **kw)


if os.environ.get("KERNEL_LDW_OPT") == "1":
    _bass_utils.run_command = _run_command_ldwopt


def build_program(slot_tiles: tuple[int, ...]) -> "bass.Bass":
    """Build the SPMD Bass program for per-core slot tile counts."""
    TT = sum(slot_tiles)
    EC = TT * TILE
    assert TT % (GROUP // TILE) == 0
    NGRP = TT // (GROUP // TILE)

    # per-tile slot id / first / last flags
    slot_of, first_of, last_of = [], [], []
    for j, tj in enumerate(slot_tiles):
        for t in range(tj):
            slot_of.append(j)
            first_of.append(t == 0)
            last_of.append(t == tj - 1)

    f32, bf16 = mybir.dt.float32, mybir.dt.bfloat16
    nc = bass.Bass("TRN2", target_bir_lowering=False, debug=False, num_devices=NCORES)

    xt_d = nc.dram_tensor("xt", [H, EC], bf16, kind="ExternalInput").ap()
    wvs_d = nc.dram_tensor("wvs", [H, 2 * H], bf16, kind="ExternalInput").ap()
    w1_d = nc.dram_tensor("w1", [S * H, H], bf16, kind="ExternalInput").ap()
    w2_d = nc.dram_tensor("w2", [H, H], bf16, kind="ExternalInput").ap()
    b1_d = nc.dram_tensor("b1", [NG, H], f32, kind="ExternalInput").ap()
    b2_d = nc.dram_tensor("b2", [H, 1], f32, kind="ExternalInput").ap()
    npad_d = nc.dram_tensor("npad", [128, NG], f32, kind="ExternalInput").ap()
    ident_d = nc.dram_tensor("ident", [128, 128], bf16, kind="ExternalInput").ap()
    qsel_d = nc.dram_tensor("qsel", [128, NG], bf16, kind="ExternalInput").ap()
    outT_d = nc.dram_tensor("outT", [H, NG], f32, kind="ExternalOutput").ap()

    with tile.TileContext(nc) as tc, ExitStack() as ctx:
        const = ctx.enter_context(tc.tile_pool(name="const", bufs=1))
        w2_sb = const.tile([128, 2 * H], bf16)
        wvs_sb = const.tile([128, 2 * 2 * H], bf16)  # k-tile k: [wv_k | ws_k]
        w1_sb = const.tile([128, 64 * H], bf16)
        ident_sb = const.tile([128, 128], bf16)
        qsel_sb = const.tile([128, NG], bf16)
        b1_sb = const.tile([NG, H], f32)
        b2_sb = const.tile([128, 2], f32)
        npad_sb = const.tile([128, NG], f32)
        P2 = const.tile([128, 64 * NG], bf16)

        for k in range(2):
            r = slice(k * 128, (k + 1) * 128)
            nc.scalar.dma_start(wvs_sb[:, k * 2 * H : (k + 1) * 2 * H], wvs_d[r, :])
        nc.scalar.dma_start(npad_sb[:], npad_d[:])
        for k in range(2):
            r = slice(k * 128, (k + 1) * 128)
            nc.scalar.dma_start(w2_sb[:, k * H : (k + 1) * H], w2_d[r, :])
            nc.scalar.dma_start(b2_sb[:, k : k + 1], b2_d[r, :])
        nc.scalar.dma_start(ident_sb[:], ident_d[:])
        nc.scalar.dma_start(qsel_sb[:], qsel_d[:])
        nc.scalar.dma_start(b1_sb[:], b1_d[:])

        # Warm the ACT function tables while the first DMAs are in flight,
        # so the table loads are off the critical path.
        warm = const.tile([1, 2], f32)
        nc.gpsimd.memset(warm[:, 0:1], 0.0)
        nc.scalar.activation(warm[:, 1:2], warm[:, 0:1], AF.Exp)
        nc.scalar.activation(warm[:, 1:2], warm[:, 0:1], AF.Sigmoid)

        # ---- main edge loop ---------------------------------------------
        xt_pool = ctx.enter_context(tc.tile_pool(name="xtp", bufs=4))
        ex_pool = ctx.enter_context(tc.tile_pool(name="exp", bufs=6))
        ext_pool = ctx.enter_context(tc.tile_pool(name="ext", bufs=2))

        NRING = 6
        vs_ring = [const.tile([128, 258], bf16, name=f"vsring{i}") for i in range(NRING)]
        for t in vs_ring:
            nc.vector.memset(t[:, 128:129], 1.0)
            nc.vector.memset(t[:, 257:258], 1.0)

        pooled_tiles: list = [None, None]

        def emit_pooled(sl, fi, la, ex, vs):
            if fi:
                pooled_tiles[0] = pl_pool.tile([128, 129], f32, tag="pl0", name=f"pl0_s{sl}")
                pooled_tiles[1] = pl_pool.tile([128, 129], f32, tag="pl1", name=f"pl1_s{sl}")
            for m in range(2):
                nc.tensor.matmul(
                    pooled_tiles[m][:],
                    ex[:, m * 128 : (m + 1) * 128],
                    vs[:, m * 129 : m * 129 + 129],
                    start=fi,
                    stop=la,
                )
            if la:
                extract_graph(sl, pooled_tiles)

        P2v = P2[:].rearrange("p (s x) -> p s x", x=2 * NG)

        def extract_graph(g, ptiles):
            copy_eng = nc.vector if g == NG - 1 else nc.gpsimd
            for m in range(2):
                den = ext_pool.tile([128, 1], f32, tag="den", name=f"den{g}_{m}")
                nc.vector.tensor_scalar_sub(
                    den[:], ptiles[m][:, 128:129], npad_sb[:, g : g + 1]
                )
                rec = ext_pool.tile([128, 1], f32, tag="rec", name=f"rec{g}_{m}")
                nc.vector.reciprocal(rec[:], den[:])
                pn = ext_pool.tile([128, 128], f32, tag="pn", name=f"pn{g}_{m}")
                nc.vector.tensor_scalar_mul(pn[:], ptiles[m][:, 0:128], rec[:])
                pt = ext_pool.tile([128, 128], f32, tag="pt", name=f"pt{g}_{m}")
                nc.vector.transpose(pt[:], pn[:])
                for hh in range(4):
                    rr = slice(hh * 32, (hh + 1) * 32)
                    src = pt[rr, hh * 32 : (hh + 1) * 32].rearrange(
                        "p (a o) -> p a o", o=1
                    )
                    copy_eng.tensor_copy(P2v[rr, :, m * NG + g : m * NG + g + 1], src)

        with (
            tc.tile_pool(name="vscp", bufs=3, space="PSUM") as vsc_pool,
            tc.tile_pool(name="plp", bufs=2, space="PSUM") as pl_pool,
        ):
            from collections import deque

            pending = deque()
            tidx = 0
            for grp in range(NGRP):
                xt = [
                    xt_pool.tile([128, GROUP], bf16, tag="xt", name=f"xt_{grp}_{i}")
                    for i in range(2)
                ]
                for k in range(2):
                    nc.sync.dma_start(
                        xt[k][:],
                        xt_d[k * 128 : (k + 1) * 128, grp * GROUP : (grp + 1) * GROUP],
                    )
                for sub in range(4):
                    sl, fi, la = slot_of[tidx], first_of[tidx], last_of[tidx]
                    e0 = sub * TILE
                    vsc = vsc_pool.tile([128, 512], f32, tag="vsc", name=f"vsc{tidx}")
                    for k in range(2):
                        nc.tensor.matmul(
                            vsc[:],
                            xt[k][:, e0 : e0 + TILE],
                            wvs_sb[:, k * 2 * H : (k + 1) * 2 * H],
                            start=(k == 0),
                            stop=(k == 1),
                        )
                    ex = ex_pool.tile([128, 256], bf16, tag="ex", name=f"ex{tidx}")
                    nc.scalar.activation(ex[:], vsc[:, H : 2 * H], AF.Exp)
                    vs = vs_ring[tidx % NRING]
                    nc.vector.tensor_copy(
                        vs[:].rearrange("p (b c) -> p b c", c=129)[:, :, 0:128],
                        vsc[:, 0:H].rearrange("p (b c) -> p b c", c=128),
                    )
                    pending.append((sl, fi, la, ex, vs))
                    while len(pending) > 2:
                        emit_pooled(*pending.popleft())
                    tidx += 1
            while pending:
                emit_pooled(*pending.popleft())

        # w1 load — one big blocked DMA on the Scalar HWDGE ring, so its 4MB
        # transfer cannot queue ahead of the edge-loop xt groups on the Sync
        # ring (the scheduler hoists it regardless of trace position).
        nc.scalar.dma_start(
            w1_sb[:].rearrange("p (k c) -> p k c", c=H),
            w1_d[:].rearrange("(k p) c -> p k c", p=128),
        )

        # ---- MLP tail ----------------------------------------------------
        with (
            tc.tile_pool(name="mlpp", bufs=2, space="PSUM") as mp,
            tc.tile_pool(name="mlps", bufs=2) as ms,
        ):
            h1pp = mp.tile([128, H], f32, tag="h1pp")
            for j in range(64):
                q = j % 4
                nc.tensor.matmul(
                    h1pp[q * 32 : q * 32 + NG, :],
                    P2[:, j * NG : (j + 1) * NG],
                    w1_sb[:, j * H : (j + 1) * H],
                    start=(j < 4),
                    stop=(j >= 60),
                    tile_position=(0, q * 32),
                    skip_group_check=True,
                )
            h1ps = ms.tile([128, H], bf16, tag="h1ps")
            nc.gpsimd.memset(h1ps[:], 0.0)
            for q in range(4):
                eng = nc.vector if q % 2 == 0 else nc.scalar
                if eng is nc.vector:
                    nc.vector.tensor_copy(
                        h1ps[q * 32 : q * 32 + NG, :], h1pp[q * 32 : q * 32 + NG, :]
                    )
                else:
                    nc.scalar.activation(
                        h1ps[q * 32 : q * 32 + NG, :],
                        h1pp[q * 32 : q * 32 + NG, :],
                        AF.Copy,
                    )
            h1p = mp.tile([NG, H], f32, tag="h1p")
            nc.tensor.matmul(h1p[:], qsel_sb[:], h1ps[:], start=True, stop=True)
            h1s = ms.tile([NG, H], f32, tag="h1s")
            nc.vector.tensor_add(h1s[:], h1p[:], b1_sb[:])
            h1g = ms.tile([NG, H], f32, tag="h1g")
            nc.scalar.activation(h1g[:], h1s[:], AF.Sigmoid)
            h1b = ms.tile([NG, H], bf16, tag="h1b")
            nc.vector.tensor_mul(h1b[:], h1s[:], h1g[:])
            h1t = []
            for m in range(2):
                h1tp = mp.tile([128, NG], bf16, tag="h1tp", name=f"h1tp{m}")
                nc.tensor.transpose(
                    h1tp[:], h1b[:, m * 128 : (m + 1) * 128], ident_sb[0:NG, 0:NG]
                )
                ht = ms.tile([128, NG], bf16, tag=f"h1t{m}")
                nc.vector.tensor_copy(ht[:], h1tp[:])
                h1t.append(ht)
            for m in range(2):
                otp = mp.tile([128, NG], f32, tag="otp", name=f"otp{m}")
                for k in range(2):
                    nc.tensor.matmul(
                        otp[:],
                        w2_sb[:, k * H + m * 128 : k * H + m * 128 + 128],
                        h1t[k][:],
                        start=(k == 0),
                        stop=(k == 1),
                    )
                osb = ms.tile([128, NG], f32, tag="osb", name=f"osb{m}")
                nc.vector.tensor_scalar_add(osb[:], otp[:], b2_sb[:, m : m + 1])
                nc.sync.dma_start(outT_d[m * 128 : (m + 1) * 128, :], osb[:])

    return nc


def get_program(slot_tiles: tuple[int, ...]) -> "bass.Bass":
    if slot_tiles not in _PROGRAM_CACHE:
        nc = build_program(slot_tiles)
        # HW-path only (CoreSim snapshots the program before this pass)
        _fix_excess_waits(nc)
        _PROGRAM_CACHE[slot_tiles] = nc
    return _PROGRAM_CACHE[slot_tiles]


# ---------------------------------------------------------------------------
# Host-side sharding / padding


def plan_shards(batch: np.ndarray):
    """Returns (assign [NCORES][NG] graph ids, slot_tiles tuple, sizes)."""
    sizes = np.bincount(batch, minlength=B).astype(np.int64)
    order = np.argsort(-sizes, kind="stable")
    assign = [[] for _ in range(NCORES)]
    for r in range(NG):
        row = order[r * NCORES : (r + 1) * NCORES]
        if r % 2 == 1:
            row = row[::-1]
        for c in range(NCORES):
            assign[c].append(int(row[c]))
    for c in range(NCORES):
        assign[c].sort(key=lambda g: -sizes[g])
    slot_tiles = []
    for j in range(NG):
        mx = max(sizes[assign[c][j]] for c in range(NCORES))
        slot_tiles.append(int(max(1, -(-mx // TILE))))
    # round total tiles up to a GROUP multiple (pad goes to the last slot)
    rem = (-sum(slot_tiles)) % (GROUP // TILE)
    slot_tiles[-1] += rem
    return assign, tuple(slot_tiles), sizes


def make_in_maps(edge_features, batch, seed_vectors, w_q, w_k, w_v, w1, b1, w2, b2):
    edge_features = np.asarray(edge_features, dtype=np.float32)
    batch = np.asarray(batch)
    assign, slot_tiles, sizes = plan_shards(batch)
    TT = sum(slot_tiles)
    EC = TT * TILE

    starts = np.searchsorted(batch, np.arange(B))
    xb = edge_features.astype(BF16)

    # Ws[hin, h*S+s] = sum_d w_k[hin, h*HD+d] * q[s, h, d] / sqrt(HD)
    q = (np.asarray(seed_vectors, np.float32) @ np.asarray(w_q, np.float32)).reshape(
        S, NH, HD
    )
    wk3 = np.asarray(w_k, np.float32).reshape(H, NH, HD)
    Ws = (np.einsum("ihd,shd->ihs", wk3, q) * SCALE).reshape(H, NH * S)
    wvs = np.concatenate([np.asarray(w_v, np.float32), Ws], axis=1)

    shared = {
        "wvs": np.ascontiguousarray(wvs.astype(BF16)),
        "w1": np.ascontiguousarray(np.asarray(w1).astype(BF16)),
        "w2": np.ascontiguousarray(np.asarray(w2).astype(BF16)),
        "b1": np.ascontiguousarray(
            np.broadcast_to(np.asarray(b1, dtype=np.float32), (NG, H))
        ),
        "b2": np.ascontiguousarray(np.asarray(b2, dtype=np.float32).reshape(H, 1)),
        "ident": np.eye(128, dtype=BF16),
        "qsel": np.ascontiguousarray(
            (np.arange(128)[:, None] % 32 == np.arange(NG)[None, :]).astype(BF16)
        ),
    }

    in_maps = []
    for c in range(NCORES):
        xt = np.zeros((H, EC), dtype=BF16)
        npad = np.zeros(NG, dtype=np.float32)
        off = 0
        for j, g in enumerate(assign[c]):
            n = int(sizes[g])
            xt[:, off : off + n] = xb[starts[g] : starts[g] + n].T
            npad[j] = slot_tiles[j] * TILE - n
            off += slot_tiles[j] * TILE
        m = dict(shared)
        m["xt"] = xt
        m["npad"] = np.ascontiguousarray(np.broadcast_to(npad, (128, NG)))
        in_maps.append(m)
    return in_maps, assign, slot_tiles


def kernel(
    edge_features,
    edge_coords,
    batch,
    seed_vectors,
    w_q,
    w_k,
    w_v,
    w1,
    b1,
    w2,
    b2,
):
    in_maps, assign, slot_tiles = make_in_maps(
        edge_features, batch, seed_vectors, w_q, w_k, w_v, w1, b1, w2, b2
    )
    nc = get_program(slot_tiles)

    res = run_bass_kernel_spmd(nc, in_maps, core_ids=list(range(NCORES)))
    global LAST_RESULTS
    LAST_RESULTS = res

    out = np.zeros((B, H), dtype=np.float32)
    for c in range(NCORES):
        outT = res.results[c]["outT"]  # [H, NG]
        for j, g in enumerate(assign[c]):
            out[g, :] = outT[:, j]
    return out



# revision 3
# speedup vs baseline: 1.1433x; 1.1433x over previous
"""AttentionPooling (ragged graph cross-attention pooling) on 8 TRN2 NeuronCores.

Strategy (SPMD, no collectives):
  * Host assigns 8 whole graphs to each of the 8 cores (serpentine by size),
    sorts each core's graphs by size into 8 "slots".  Slot j has a fixed tile
    count T[j] (shared by all cores, since the instruction stream is shared);
    each graph's edges are placed at its slot offset and zero-padded.
  * Host ships x^T (transposed edge features, bf16) per core + replicated
    weights.  Padding edges give exp(0)=1 in the softmax denominator, which is
    corrected with a host-computed per-slot pad count.
  * Softmax is computed without max-subtraction (scores ~ N(0,1); exp cannot
    overflow fp32) — mathematically identical to the reference's stable form.
  * Scores are linear in x: scores = (x @ w_k) . q  =  x @ Ws where
    Ws[:, (h,s)] = sum_d w_k[:, (h,d)] q[s,h,d] / sqrt(hd).
  * v2 (vs the prior version): the whole x^T stream is preloaded into SBUF
    with a few large column-chunk DMAs (4-8KB per-partition packets) issued
    up-front on the Sync HWDGE ring, so the PE never waits on DMA mid-loop
    (PE idle gaps > 3.4us re-throttle the HAM clock gate to 1.2 GHz).  w1 is
    host-transposed to [128, 16384] and streamed on the same ring after the
    x chunks.  Scores and v are computed as separate N=256 matmuls into
    per-PAIR PSUM tiles so the exp (ACT) and the v PSUM->SBUF cast (DVE) are
    amortized over 512 columns per instruction.  A short burst of junk
    matmuls warms the HAM clock while the first chunk DMA is in flight.
  * Per graph: denom -= npad; normalize by 1/denom (DVE); 32x32 block
    transpose (DVE StreamTranspose) to build the [128, (s,half)*8graphs]
    operand P2 for the MLP (w1 needs no permutation in this layout).
  * MLP: h1 = silu(pooled @ w1 + b1) (PE, 4-way tile_position-packed), with
    sigmoid computed via the already-resident Exp table (1/(1+e^-x)) to
    avoid a ~2.7us ACT table switch; out = h1 @ w2 + b2 (PE), emitted as
    out^T [256, 8] per core; the host scatters core outputs into [64, 256].
"""

import os
import sys
from collections import deque
from contextlib import ExitStack

import numpy as np

for _p in ("/opt/trn_rl_repo",):
    if _p not in sys.path:
        sys.path.append(_p)

import ml_dtypes  # noqa: E402

import concourse.bass as bass  # noqa: E402
import concourse.tile as tile  # noqa: E402
from concourse import mybir  # noqa: E402
from concourse.bass_utils import run_bass_kernel_spmd  # noqa: E402
from concourse.vector_clock import ScopedClock  # noqa: E402

BF16 = ml_dtypes.bfloat16

E, B, H, S, NH, HD = 131072, 64, 256, 32, 8, 32
NCORES = 8
NG = B // NCORES        # graphs (slots) per core
TILE = 128              # edge tile
SCALE = 1.0 / float(np.sqrt(HD))
CHUNK0_TILES = 8        # first xt chunk (small, for fast PE start)
CHUNK_TILES = 32        # steady-state xt chunk size (tiles, even)
N_WARM_MM = 18          # junk matmuls to warm the HAM clock gate

AF = mybir.ActivationFunctionType

# ---------------------------------------------------------------------------
# Walrus workaround: this toolchain's InstDrain accepts only ONE sync wait;
# Tile's kernel-tail drain carries one wait per outstanding semaphore.
# Split it into a chain of single-wait drains.
_MAXW = 1


def _split_drain_and_barrier(self, tick_clock, wait_clock):
    nc = self.nc
    drain_inst = nc.sync.drain()
    wait_clock.add_sem_waits(
        drain_inst.ins, ScopedClock({None: tick_clock.global_clock})
    )
    waits = list(drain_inst.ins.sync_info.on_wait)
    if len(waits) > _MAXW:
        drain_inst.ins.sync_info = mybir.SyncInfo(on_wait=waits[:_MAXW], on_update=[])
        for i in range(_MAXW, len(waits), _MAXW):
            d2 = nc.sync.drain()
            d2.ins.sync_info = mybir.SyncInfo(
                on_wait=waits[i : i + _MAXW], on_update=[]
            )
    nc.all_engine_barrier()
    popped = nc._tile_sem_poison_stack.pop()
    assert popped is self._sem_poison
    nc.clear_and_free_semaphores(list(self.sems.allocated().values()))
    nc.all_engine_barrier()


tile.TileContext._drain_and_barrier = _split_drain_and_barrier

# Engine instructions are capped at 2 sync waits by this walrus (Drain/NoOp
# at 1).  Tile's sem-assignment occasionally emits more.  Hoist the excess
# onto single-wait NoOps inserted just before, on the same engine — the
# engine stalls at the NoOp instead, which is semantically identical.
_WAIT_CAP = {"InstDrain": 1}
_WAIT_CAP_DEFAULT = 1


def _fix_excess_waits(nc):
    n_fixed = 0
    for fn in nc.m.functions:
        for bb in fn.blocks:
            insts = bb.instructions
            out = []
            changed = False
            for inst in insts:
                si = inst.sync_info
                waits = list(si.on_wait) if si is not None else []
                cap = _WAIT_CAP.get(type(inst).__name__, _WAIT_CAP_DEFAULT)
                if len(waits) > cap:
                    changed = True
                    n_fixed += 1
                    excess = waits[: len(waits) - cap]
                    for i, w in enumerate(excess):
                        nop = mybir.InstNoOp(
                            name=f"{inst.name}-hw{i}", ins=[], outs=[]
                        )
                        nop.engine = inst.engine
                        nop.sync_info = mybir.SyncInfo(on_wait=[w], on_update=[])
                        out.append(nop)
                    inst.sync_info = mybir.SyncInfo(
                        on_wait=waits[len(excess) :], on_update=list(si.on_update)
                    )
                out.append(inst)
            if changed:
                bb.instructions = out
    return n_fixed

# ---------------------------------------------------------------------------

_PROGRAM_CACHE: dict[tuple, "bass.Bass"] = {}
LAST_RESULTS = None  # BassKernelResults of the most recent run (for testing)


def _install_ntff_hook_shim():
    """The image's antenv lacks axon_hooks; recreate it so trace=True works."""
    try:
        import types

        import antenv

        if "antenv.axon_hooks" not in sys.modules:
            mod = types.ModuleType("antenv.axon_hooks")
            mod._hook = None

            def set_axon_ntff_profile_hook(h):
                mod._hook = h

            def get_axon_ntff_profile_hook():
                return mod._hook

            mod.set_axon_ntff_profile_hook = set_axon_ntff_profile_hook
            mod.get_axon_ntff_profile_hook = get_axon_ntff_profile_hook
            sys.modules["antenv.axon_hooks"] = mod
            antenv.axon_hooks = mod
        import antenv.axon_hooks as ah

        if ah.get_axon_ntff_profile_hook() is None:
            from trn_agent_boot.trn_boot import _ntff_profile_via_ctypes

            ah.set_axon_ntff_profile_hook(
                _ntff_profile_via_ctypes("/opt/axon/libaxon_pjrt.so")
            )
    except Exception:
        pass


_install_ntff_hook_shim()

# Optional experiment: let walrus double-buffer LDWEIGHTS (default off here).
import concourse.bass_utils as _bass_utils  # noqa: E402

_orig_run_command = _bass_utils.run_command


def _run_command_ldwopt(cmd, **kw):
    if isinstance(cmd, list):
        cmd = [
            "--enable-ldw-opt=true" if c == "--enable-ldw-opt=false" else c
            for c in cmd
        ]
    return _orig_run_command(cmd, **kw)


if os.environ.get("KERNEL_LDW_OPT") == "1":
    _bass_utils.run_command = _run_command_ldwopt


def _chunk_bounds(TT: int) -> list[tuple[int, int]]:
    """Even-sized tile chunks: small first chunk, then CHUNK_TILES."""
    bounds = []
    t = 0
    first = True
    while t < TT:
        n = CHUNK0_TILES if first else CHUNK_TILES
        first = False
        n = min(n, TT - t)
        bounds.append((t, t + n))
        t += n
    return bounds


def build_program(slot_tiles: tuple[int, ...]) -> "bass.Bass":
    """Build the SPMD Bass program for per-core slot tile counts."""
    TT = sum(slot_tiles)
    assert TT % 2 == 0
    EC = TT * TILE
    chunks = _chunk_bounds(TT)
    chunk_of = []
    for ci, (a, b) in enumerate(chunks):
        chunk_of += [ci] * (b - a)

    # per-tile slot id / first / last flags
    slot_of, first_of, last_of = [], [], []
    for j, tj in enumerate(slot_tiles):
        for t in range(tj):
            slot_of.append(j)
            first_of.append(t == 0)
            last_of.append(t == tj - 1)

    f32, bf16 = mybir.dt.float32, mybir.dt.bfloat16
    nc = bass.Bass("TRN2", target_bir_lowering=False, debug=False, num_devices=NCORES)

    xt_d = nc.dram_tensor("xt", [H, EC], bf16, kind="ExternalInput").ap()
    wvs_d = nc.dram_tensor("wvs", [H, 2 * H], bf16, kind="ExternalInput").ap()
    w1p_d = nc.dram_tensor("w1p", [128, 64 * H], bf16, kind="ExternalInput").ap()
    w2_d = nc.dram_tensor("w2", [H, H], bf16, kind="ExternalInput").ap()
    b1_d = nc.dram_tensor("b1", [NG, H], f32, kind="ExternalInput").ap()
    b2_d = nc.dram_tensor("b2", [H, 1], f32, kind="ExternalInput").ap()
    npad_d = nc.dram_tensor("npad", [128, NG], f32, kind="ExternalInput").ap()
    ident_d = nc.dram_tensor("ident", [128, 128], bf16, kind="ExternalInput").ap()
    qsel_d = nc.dram_tensor("qsel", [128, NG], bf16, kind="ExternalInput").ap()
    outT_d = nc.dram_tensor("outT", [H, NG], f32, kind="ExternalOutput").ap()

    with tile.TileContext(nc) as tc, ExitStack() as ctx:
        const = ctx.enter_context(tc.tile_pool(name="const", bufs=1))
        # k-tile k of [wv_k | ws_k]: wvs_sb[:, k*512 : k*512+256] = wv_k,
        #                            wvs_sb[:, k*512+256 : (k+1)*512] = ws_k
        wvs_sb = const.tile([128, 2 * 2 * H], bf16)
        w2_sb = const.tile([128, 2 * H], bf16)
        ident_sb = const.tile([128, 128], bf16)
        qsel_sb = const.tile([128, NG], bf16)
        b1_sb = const.tile([NG, H], f32)
        b2_sb = const.tile([128, 2], f32)
        npad_sb = const.tile([128, NG], f32)
        P2 = const.tile([128, 64 * NG], bf16)

        # small replicated weights on the Scalar HWDGE ring (land in ~1us)
        for k in range(2):
            r = slice(k * 128, (k + 1) * 128)
            nc.scalar.dma_start(wvs_sb[:, k * 2 * H : (k + 1) * 2 * H], wvs_d[r, :])
        nc.scalar.dma_start(npad_sb[:], npad_d[:])
        for k in range(2):
            r = slice(k * 128, (k + 1) * 128)
            nc.scalar.dma_start(w2_sb[:, k * H : (k + 1) * H], w2_d[r, :])
            nc.scalar.dma_start(b2_sb[:, k : k + 1], b2_d[r, :])
        nc.scalar.dma_start(ident_sb[:], ident_d[:])
        nc.scalar.dma_start(qsel_sb[:], qsel_d[:])
        nc.scalar.dma_start(b1_sb[:], b1_d[:])

        # Warm the ACT exp table while the first DMAs are in flight.
        warm = const.tile([1, 2], f32)
        nc.gpsimd.memset(warm[:, 0:1], 0.0)
        nc.scalar.activation(warm[:, 1:2], warm[:, 0:1], AF.Exp)

        # x^T stream: big column-chunk DMAs on the Sync HWDGE ring (FIFO).
        # All issued up-front, dep-free -> they execute in emission order and
        # stream at full engine rate; w1 pieces queue behind them.
        xc = [[], []]
        for ci, (a, b) in enumerate(chunks):
            for k in range(2):
                t_ = const.tile([128, (b - a) * TILE], bf16, name=f"xc{k}_{ci}")
                xc[k].append(t_)
                nc.sync.dma_start(
                    t_[:], xt_d[k * 128 : (k + 1) * 128, a * TILE : b * TILE]
                )
        # w1 (host-pretransposed [128, 64*H]): 4 pieces, behind the x chunks
        NW1 = 4
        w1c = []
        w1w = (64 * H) // NW1
        for i in range(NW1):
            t_ = const.tile([128, w1w], bf16, name=f"w1c{i}")
            w1c.append(t_)
            nc.sync.dma_start(t_[:], w1p_d[:, i * w1w : (i + 1) * w1w])

        def w1_block(j):  # [128, H] slice for MLP k-chunk j (j = 2s+m)
            per = w1w // H
            return w1c[j // per][:, (j % per) * H : (j % per + 1) * H]

        # HAM warm-up: junk matmuls on the first-loaded weight tile keep the
        # PE busy while chunk 0 is in flight, so real tiles start at 2.4 GHz.
        with tc.tile_pool(name="warmp", bufs=1, space="PSUM") as wp:
            wps = wp.tile([128, 512], f32)
            for i in range(N_WARM_MM):
                nc.tensor.matmul(
                    wps[:], wvs_sb[:, 0:128], wvs_sb[:, 0:512],
                    start=True, stop=True,
                )

        # ---- main edge loop (pairs of tiles) ----------------------------
        NRING = 4
        vs_ring = [const.tile([128, 4 * 129], bf16, name=f"vsring{i}") for i in range(NRING)]
        for t in vs_ring:
            for blk in range(4):
                nc.vector.memset(t[:, blk * 129 + 128 : blk * 129 + 129], 1.0)

        ex_pool = ctx.enter_context(tc.tile_pool(name="exp", bufs=4))
        ext_pool = ctx.enter_context(tc.tile_pool(name="ext", bufs=2))

        pooled_cur = [None, None]
        P2v = P2[:].rearrange("p (s x) -> p s x", x=2 * NG)

        def extract_graph(g, ptiles):
            copy_eng = nc.vector if g == NG - 1 else nc.gpsimd
            for m in range(2):
                pl = ptiles[m]
                den = ext_pool.tile([128, 1], f32, tag="den", name=f"den{g}_{m}")
                nc.vector.tensor_scalar_sub(
                    den[:], pl[:, 128:129], npad_sb[:, g : g + 1]
                )
                rec = ext_pool.tile([128, 1], f32, tag="rec", name=f"rec{g}_{m}")
                nc.vector.reciprocal(rec[:], den[:])
                pn = ext_pool.tile([128, 128], f32, tag="pn", name=f"pn{g}_{m}")
                nc.vector.tensor_scalar_mul(pn[:], pl[:, 0:128], rec[:])
                pt = ext_pool.tile([128, 128], f32, tag="pt", name=f"pt{g}_{m}")
                nc.vector.transpose(pt[:], pn[:])
                for hh in range(4):
                    rr = slice(hh * 32, (hh + 1) * 32)
                    src = pt[rr, hh * 32 : (hh + 1) * 32].rearrange(
                        "p (a o) -> p a o", o=1
                    )
                    copy_eng.tensor_copy(P2v[rr, :, m * NG + g : m * NG + g + 1], src)

        def emit_pool(tl, tg, ex2, vs):
            sl, fi, la = slot_of[tg], first_of[tg], last_of[tg]
            if fi:
                pooled_cur[0] = pl_pool.tile(
                    [128, 129], f32, tag="pl0", name=f"pl0_s{sl}"
                )
                pooled_cur[1] = pl_pool.tile(
                    [128, 129], f32, tag="pl1", name=f"pl1_s{sl}"
                )
            for m in range(2):
                nc.tensor.matmul(
                    pooled_cur[m][:],
                    ex2[:, tl * 256 + m * 128 : tl * 256 + m * 128 + 128],
                    vs[:, (2 * tl + m) * 129 : (2 * tl + m) * 129 + 129],
                    start=fi,
                    stop=la,
                )
            if la:
                extract_graph(sl, pooled_cur)

        with (
            tc.tile_pool(name="scp", bufs=2, space="PSUM") as sc_pool,
            tc.tile_pool(name="vp", bufs=2, space="PSUM") as v_pool,
            tc.tile_pool(name="plp", bufs=2, space="PSUM") as pl_pool,
        ):
            pending = deque()
            for p in range(TT // 2):
                t0 = 2 * p
                ci = chunk_of[t0]
                c0 = chunks[ci][0]
                sc2 = sc_pool.tile([128, 512], f32, tag="sc", name=f"sc{p}")
                v2 = v_pool.tile([128, 512], f32, tag="v", name=f"v{p}")
                for tl in range(2):
                    off = (t0 + tl - c0) * TILE
                    for k in range(2):
                        lhsT = xc[k][ci][:, off : off + TILE]
                        nc.tensor.matmul(
                            sc2[:, tl * 256 : (tl + 1) * 256],
                            lhsT,
                            wvs_sb[:, k * 512 + 256 : k * 512 + 512],
                            start=(k == 0),
                            stop=(k == 1),
                        )
                        nc.tensor.matmul(
                            v2[:, tl * 256 : (tl + 1) * 256],
                            lhsT,
                            wvs_sb[:, k * 512 : k * 512 + 256],
                            start=(k == 0),
                            stop=(k == 1),
                        )
                ex2 = ex_pool.tile([128, 512], bf16, tag="ex", name=f"ex{p}")
                nc.scalar.activation(ex2[:], sc2[:], AF.Exp)
                vs = vs_ring[p % NRING]
                nc.vector.tensor_copy(
                    vs[:].rearrange("p (b c) -> p b c", c=129)[:, :, 0:128],
                    v2[:].rearrange("p (b c) -> p b c", c=128),
                )
                pending.append((p, ex2, vs))
                while len(pending) > 1:
                    q, exq, vsq = pending.popleft()
                    emit_pool(0, 2 * q, exq, vsq)
                    emit_pool(1, 2 * q + 1, exq, vsq)
            while pending:
                q, exq, vsq = pending.popleft()
                emit_pool(0, 2 * q, exq, vsq)
                emit_pool(1, 2 * q + 1, exq, vsq)

        # ---- MLP tail ----------------------------------------------------
        with (
            tc.tile_pool(name="mlpp", bufs=2, space="PSUM") as mp,
            tc.tile_pool(name="mlps", bufs=2) as ms,
        ):
            h1pp = mp.tile([128, H], f32, tag="h1pp")
            for j in range(64):
                q = j % 4
                nc.tensor.matmul(
                    h1pp[q * 32 : q * 32 + NG, :],
                    P2[:, j * NG : (j + 1) * NG],
                    w1_block(j),
                    start=(j < 4),
                    stop=(j >= 60),
                    tile_position=(0, q * 32),
                    skip_group_check=True,
                )
            h1ps = ms.tile([128, H], bf16, tag="h1ps")
            nc.gpsimd.memset(h1ps[:], 0.0)
            for q in range(4):
                if q % 2 == 0:
                    nc.vector.tensor_copy(
                        h1ps[q * 32 : q * 32 + NG, :], h1pp[q * 32 : q * 32 + NG, :]
                    )
                else:
                    nc.scalar.activation(
                        h1ps[q * 32 : q * 32 + NG, :],
                        h1pp[q * 32 : q * 32 + NG, :],
                        AF.Copy,
                    )
            h1p = mp.tile([NG, H], f32, tag="h1p")
            nc.tensor.matmul(h1p[:], qsel_sb[:], h1ps[:], start=True, stop=True)
            h1s = ms.tile([NG, H], f32, tag="h1s")
            nc.vector.tensor_add(h1s[:], h1p[:], b1_sb[:])
            # silu via the resident Exp table: x * 1/(1 + exp(-x))
            h1e = ms.tile([NG, H], f32, tag="h1e")
            nc.scalar.activation(h1e[:], h1s[:], AF.Exp, scale=-1.0)
            h1a = ms.tile([NG, H], f32, tag="h1a")
            nc.vector.tensor_scalar_add(h1a[:], h1e[:], 1.0)
            h1r = ms.tile([NG, H], f32, tag="h1r")
            nc.vector.reciprocal(h1r[:], h1a[:])
            h1b = ms.tile([NG, H], bf16, tag="h1b")
            nc.vector.tensor_mul(h1b[:], h1s[:], h1r[:])
            h1t = []
            for m in range(2):
                h1tp = mp.tile([128, NG], bf16, tag="h1tp", name=f"h1tp{m}")
                nc.tensor.transpose(
                    h1tp[:], h1b[:, m * 128 : (m + 1) * 128], ident_sb[0:NG, 0:NG]
                )
                ht = ms.tile([128, NG], bf16, tag=f"h1t{m}")
                nc.vector.tensor_copy(ht[:], h1tp[:])
                h1t.append(ht)
            for m in range(2):
                otp = mp.tile([128, NG], f32, tag="otp", name=f"otp{m}")
                for k in range(2):
                    nc.tensor.matmul(
                        otp[:],
                        w2_sb[:, k * H + m * 128 : k * H + m * 128 + 128],
                        h1t[k][:],
                        start=(k == 0),
                        stop=(k == 1),
                    )
                osb = ms.tile([128, NG], f32, tag="osb", name=f"osb{m}")
                nc.vector.tensor_scalar_add(osb[:], otp[:], b2_sb[:, m : m + 1])
                nc.sync.dma_start(outT_d[m * 128 : (m + 1) * 128, :], osb[:])

    return nc


def get_program(slot_tiles: tuple[int, ...]) -> "bass.Bass":
    if slot_tiles not in _PROGRAM_CACHE:
        nc = build_program(slot_tiles)
        # HW-path only (CoreSim snapshots the program before this pass)
        _fix_excess_waits(nc)
        _PROGRAM_CACHE[slot_tiles] = nc
    return _PROGRAM_CACHE[slot_tiles]


# ---------------------------------------------------------------------------
# Host-side sharding / padding


def plan_shards(batch: np.ndarray):
    """Returns (assign [NCORES][NG] graph ids, slot_tiles tuple, sizes)."""
    sizes = np.bincount(batch, minlength=B).astype(np.int64)
    order = np.argsort(-sizes, kind="stable")
    assign = [[] for _ in range(NCORES)]
    for r in range(NG):
        row = order[r * NCORES : (r + 1) * NCORES]
        if r % 2 == 1:
            row = row[::-1]
        for c in range(NCORES):
            assign[c].append(int(row[c]))
    for c in range(NCORES):
        assign[c].sort(key=lambda g: -sizes[g])
    slot_tiles = []
    for j in range(NG):
        mx = max(sizes[assign[c][j]] for c in range(NCORES))
        slot_tiles.append(int(max(1, -(-mx // TILE))))
    # round total tiles up to an even count (pad goes to the last slot)
    rem = (-sum(slot_tiles)) % 2
    slot_tiles[-1] += rem
    return assign, tuple(slot_tiles), sizes


def make_in_maps(edge_features, batch, seed_vectors, w_q, w_k, w_v, w1, b1, w2, b2):
    edge_features = np.asarray(edge_features, dtype=np.float32)
    batch = np.asarray(batch)
    assign, slot_tiles, sizes = plan_shards(batch)
    TT = sum(slot_tiles)
    EC = TT * TILE

    starts = np.searchsorted(batch, np.arange(B))
    xb = edge_features.astype(BF16)

    # Ws[hin, h*S+s] = sum_d w_k[hin, h*HD+d] * q[s, h, d] / sqrt(HD)
    q = (np.asarray(seed_vectors, np.float32) @ np.asarray(w_q, np.float32)).reshape(
        S, NH, HD
    )
    wk3 = np.asarray(w_k, np.float32).reshape(H, NH, HD)
    Ws = (np.einsum("ihd,shd->ihs", wk3, q) * SCALE).reshape(H, NH * S)
    wvs = np.concatenate([np.asarray(w_v, np.float32), Ws], axis=1)

    # w1 pre-transposed so the device DMA is fully contiguous:
    # w1p[p, j*H + c] = w1[j*128 + p, c]
    w1p = (
        np.asarray(w1, np.float32)
        .astype(BF16)
        .reshape(64, 128, H)
        .transpose(1, 0, 2)
        .reshape(128, 64 * H)
    )

    shared = {
        "wvs": np.ascontiguousarray(wvs.astype(BF16)),
        "w1p": np.ascontiguousarray(w1p),
        "w2": np.ascontiguousarray(np.asarray(w2).astype(BF16)),
        "b1": np.ascontiguousarray(
            np.broadcast_to(np.asarray(b1, dtype=np.float32), (NG, H))
        ),
        "b2": np.ascontiguousarray(np.asarray(b2, dtype=np.float32).reshape(H, 1)),
        "ident": np.eye(128, dtype=BF16),
        "qsel": np.ascontiguousarray(
            (np.arange(128)[:, None] % 32 == np.arange(NG)[None, :]).astype(BF16)
        ),
    }

    in_maps = []
    for c in range(NCORES):
        xt = np.zeros((H, EC), dtype=BF16)
        npad = np.zeros(NG, dtype=np.float32)
        off = 0
        for j, g in enumerate(assign[c]):
            n = int(sizes[g])
            xt[:, off : off + n] = xb[starts[g] : starts[g] + n].T
            npad[j] = slot_tiles[j] * TILE - n
            off += slot_tiles[j] * TILE
        m = dict(shared)
        m["xt"] = xt
        m["npad"] = np.ascontiguousarray(np.broadcast_to(npad, (128, NG)))
        in_maps.append(m)
    return in_maps, assign, slot_tiles


def kernel(
    edge_features,
    edge_coords,
    batch,
    seed_vectors,
    w_q,
    w_k,
    w_v,
    w1,
    b1,
    w2,
    b2,
):
    in_maps, assign, slot_tiles = make_in_maps(
        edge_features, batch, seed_vectors, w_q, w_k, w_v, w1, b1, w2, b2
    )
    nc = get_program(slot_tiles)

    res = run_bass_kernel_spmd(nc, in_maps, core_ids=list(range(NCORES)))
    global LAST_RESULTS
    LAST_RESULTS = res

    out = np.zeros((B, H), dtype=np.float32)
    for c in range(NCORES):
        outT = res.results[c]["outT"]  # [H, NG]
        for j, g in enumerate(assign[c]):
            out[g, :] = outT[:, j]
    return out
